# revision 9
# baseline (speedup 1.0000x reference)
"""DDiT block kernel v2 for 8 Trainium2 NeuronCores.

Sharding: (batch, sequence-half) -> 8 shards, as v1. Feature-major layout
(model dim on partitions). v2 changes vs v1:
  - fp8e4m3 + DoubleRow matmuls for Q/K/V projections and attention AV
    (halves those matmul counts); scores, attn-out and MLP stay bf16.
  - rope rotate-half swap via a PE permutation matmul (no SBUF->SBUF DMA).
  - attention probs stored fp8 with a *2 exp bias (cancels in the softmax
    normalization); denominator via the ones-column trick as v1.
  - consolidated tiles and DMAs (~30 DMAs total vs 130), ring-shared SBUF
    tags with in-place LN applies to fit the 208 KiB/partition budget.
"""

import numpy as np
import ml_dtypes

BF = ml_dtypes.bfloat16
F8 = ml_dtypes.float8_e4m3

B, S, D, H, HD = 4, 1024, 1024, 16, 64
Q = 512
KO = 8
MLP = 4096
LN_EPS = 1e-5
EXPC = float(np.log(2.0))   # probs scaled by 2 (cancels in normalization)

_CACHE = {}


def _build_program(repeat=1, stage="full"):
    import concourse.bass as bass
    import concourse.mybir as mybir
    import concourse.tile as tile
    from concourse import bacc

    f32 = mybir.dt.float32
    bf = mybir.dt.bfloat16
    f8 = mybir.dt.float8e4
    AF = mybir.ActivationFunctionType
    ALU = mybir.AluOpType
    DR = mybir.MatmulPerfMode.DoubleRow
    ts = bass.ts

    nc = bacc.Bacc("TRN2", target_bir_lowering=False, debug=False,
                   enable_asserts=False)

    def din(name, shape, dt=bf):
        return nc.dram_tensor(name, shape, dt, kind="ExternalInput").ap()

    xb_d = din("xb", [128, KO, S])
    xs_d = din("xskip", [128, KO, Q], f32)
    ccss_d = din("ccss", [128, 2 * S])
    bias_d = din("bias", [128, 80], f32)
    perm_d = din("permw", [128, 128])
    wq_d = din("wq8", [128, KO, D], f8)
    wk_d = din("wk8", [128, KO, D], f8)
    wv_d = din("wv8", [128, KO, D], f8)
    wo_d = din("wao", [2, 128, KO, 512])
    w1_d = din("wm1", [8, 128, KO, 512])
    w2_d = din("wm2", [8, 128, 32, 128])
    yt_d = nc.dram_tensor("yt", [128, KO, Q], f32, kind="ExternalOutput").ap()

    with tile.TileContext(nc) as tc:
        with tc.tile_pool(name="sb", bufs=1) as sb, \
             tc.tile_pool(name="ps", bufs=1, space="PSUM") as ps:
            for _rep in range(repeat):

                def psum(name="pt"):
                    return ps.tile([128, 512], f32, tag="p", bufs=4, name=name)

                def psum2(name="pt2"):
                    return ps.tile([128, 1024], f32, tag="p2", bufs=2, name=name)

                def tmpf(name="tf"):
                    return sb.tile([128, 512], f32, tag="tmpf", bufs=3, name=name)

                def scr(name="sc"):
                    return sb.tile([128, 512], bf, tag="scr", bufs=6, name=name)

                def wall(name="w"):
                    return sb.tile([128, KO, 512], bf, tag="wall", bufs=5,
                                   name=name)

                # ---- input DMAs (weights streamed through the "wall" ring) ----
                xb = sb.tile([128, KO, S], bf, tag="A", bufs=2, name="xb")
                for _c in range(4):
                    nc.sync.dma_start(xb[:, 2 * _c:2 * _c + 2, :],
                                      xb_d[:, 2 * _c:2 * _c + 2, :])
                xskip = sb.tile([128, KO, Q], f32, tag="F", bufs=2, name="xskip")
                nc.sync.dma_start(xskip[:], xs_d[:])
                ccss = sb.tile([128, 2 * S], bf, tag="ccss", bufs=1)
                nc.sync.dma_start(ccss[:], ccss_d[:])
                cc = ccss[:, 0:S]
                ss = ccss[:, S:2 * S]
                bias = sb.tile([128, 80], f32, tag="bias", bufs=1)
                nc.sync.dma_start(bias[:], bias_d[:])
                bq_s, bk_s = bias[:, 0:8], bias[:, 8:16]
                bo_s, gm_s = bias[:, 16:24], bias[:, 24:32]
                b1_s = bias[:, 32:64]
                b2_s, gp_s = bias[:, 64:72], bias[:, 72:80]
                permw = sb.tile([128, 128], bf, tag="permw", bufs=1)
                nc.sync.dma_start(permw[:], perm_d[:])
                wq8 = sb.tile([128, KO, D], f8, tag="wall", bufs=5, name="wq8")
                nc.sync.dma_start(wq8[:], wq_d[:])
                wk8 = sb.tile([128, KO, D], f8, tag="wall", bufs=5, name="wk8")
                nc.sync.dma_start(wk8[:], wk_d[:])
                wv8 = sb.tile([128, KO, D], f8, tag="wall", bufs=5, name="wv8")
                nc.sync.dma_start(wv8[:], wv_d[:])
                wao_sb = []
                for i in range(2):
                    t = wall(f"wao{i}")
                    nc.sync.dma_start(t[:], wo_d[i])
                    wao_sb.append(t)
                w1_sb = []
                for i in range(8):
                    t = wall(f"w1p{i}")
                    nc.sync.dma_start(t[:], w1_d[i])
                    w1_sb.append(t)
                w2_sb = []
                for i in range(8):
                    t = sb.tile([128, 32, 128], bf, tag="wall", bufs=5,
                                name=f"w2p{i}")
                    nc.sync.dma_start(t[:], w2_d[i])
                    w2_sb.append(t)

                ones_b = sb.tile([128, 128], bf, tag="ones", bufs=1)
                nc.vector.memset(ones_b[:], 1.0)
                # PE warmup: ~7us of dummy matmuls while input DMAs land, so
                # the HAM clock-gate reaches 2.4 GHz before real work arrives
                pwu = ps.tile([128, 512], f32, tag="p", bufs=4, name="pwu")
                for _wu in range(16):
                    nc.tensor.matmul(pwu[:], ones_b[:], ones_b[:, 0:1].broadcast_to([128, 512]),
                                     start=(_wu == 0), stop=(_wu == 15))
                eps_ap = sb.tile([128, 1], f32, tag="eps", bufs=1)
                nc.vector.memset(eps_ap[:], LN_EPS)
                ln2_ap = sb.tile([128, 1], f32, tag="ln2c", bufs=1)
                nc.vector.memset(ln2_ap[:], EXPC)

                # ---- P1: LN1 over all 1024 tokens ----
                sq = sb.tile([128, KO, S], bf, tag="A", bufs=2, name="sq")
                for _c in range(4):
                    nc.scalar.square(sq[:, 2 * _c:2 * _c + 2, :],
                                     xb[:, 2 * _c:2 * _c + 2, :])
                ps_s1 = [psum("ps1") for _ in range(2)]
                ps_s2 = [psum("ps2") for _ in range(2)]
                for ko in range(KO):
                    for tb in range(2):
                        nc.tensor.matmul(ps_s1[tb][:], ones_b[:],
                                         xb[:, ko, ts(tb, 512)],
                                         start=(ko == 0), stop=(ko == KO - 1))
                        nc.tensor.matmul(ps_s2[tb][:], ones_b[:],
                                         sq[:, ko, ts(tb, 512)],
                                         start=(ko == 0), stop=(ko == KO - 1))

                mu01 = sb.tile([128, S], bf, tag="stats16", bufs=2, name="mu01")
                rstd01 = sb.tile([128, S], bf, tag="stats16", bufs=2, name="rstd01")
                for tb in range(2):
                    mu = tmpf("mu")
                    nc.vector.tensor_scalar_mul(mu[:], ps_s1[tb][:], 1.0 / D)
                    ex2 = tmpf("ex2")
                    nc.vector.tensor_scalar_mul(ex2[:], ps_s2[tb][:], 1.0 / D)
                    var = tmpf("var")
                    nc.vector.tensor_tensor(var[:], mu[:], mu[:], ALU.mult)
                    nc.vector.tensor_tensor(ex2[:], ex2[:], var[:], ALU.subtract)
                    nc.scalar.activation(var[:], ex2[:], AF.Sqrt, bias=eps_ap[:])
                    nc.vector.tensor_copy(mu01[:, ts(tb, 512)], mu[:])
                    with nc.allow_low_precision(reason="bf16 LN rstd"):
                        nc.vector.reciprocal(rstd01[:, ts(tb, 512)], var[:])

                pka = ps.tile([128, 512], f32, tag="p", bufs=4, name="pka")
                for _ka in range(6):
                    nc.tensor.matmul(pka[:], ones_b[:], mu01[:, 0:512],
                                     start=(_ka == 0), stop=(_ka == 5))
                mu_b = mu01[:].unsqueeze(1).broadcast_to([128, 2, S])
                rstd_b = rstd01[:].unsqueeze(1).broadcast_to([128, 2, S])
                # in-place per 2-ko chunk: xb <- xb - mu ; g8 = fp8(xb * rstd)
                g8 = sb.tile([128, KO, S], f8, tag="g8", bufs=1)
                for kp in range(4):
                    c = slice(2 * kp, 2 * kp + 2)
                    nc.vector.tensor_tensor(xb[:, c, :], xb[:, c, :], mu_b,
                                            ALU.subtract)
                    with nc.allow_low_precision(reason="fp8 LN1 activations"):
                        nc.vector.tensor_tensor(g8[:, c, :], xb[:, c, :],
                                                rstd_b, ALU.mult)

                # ---- P2: projections + rope (fp8 DoubleRow) ----
                qa = sb.tile([128, KO, Q], bf, tag="B", bufs=3, name="qa")
                qr = sb.tile([128, KO, Q], bf, tag="B", bufs=3, name="qr")
                for jo in range(KO):
                    pq = psum("pq")
                    for kp in range(4):
                        nc.tensor.matmul(pq[:], wq8[:, 2 * kp:2 * kp + 2, ts(jo, 128)],
                                         g8[:, 2 * kp:2 * kp + 2, 0:Q],
                                         start=(kp == 0), stop=(kp == 3),
                                         perf_mode=DR)
                    nc.scalar.add(qa[:, jo, :], pq[:], bq_s[:, jo:jo + 1])
                    psw = psum("psw")
                    nc.tensor.matmul(psw[:], permw[:], qa[:, jo, :],
                                     start=True, stop=True)
                    s2 = scr("s2q")
                    nc.vector.tensor_tensor(s2[:], psw[:], ss[:, 0:Q], ALU.mult)
                    s1 = scr("s1q")
                    nc.vector.tensor_tensor(s1[:], qa[:, jo, :], cc[:, 0:Q],
                                            ALU.mult)
                    nc.vector.tensor_tensor(qr[:, jo, :], s1[:], s2[:], ALU.add)

                ka = sb.tile([128, KO, S], bf, tag="A", bufs=2, name="ka")
                kr = sb.tile([128, KO, S], bf, tag="A", bufs=2, name="kr")
                for jo in range(KO):
                    for tb in range(2):
                        pk = psum("pk")
                        for kp in range(4):
                            nc.tensor.matmul(pk[:], wk8[:, 2 * kp:2 * kp + 2, ts(jo, 128)],
                                             g8[:, 2 * kp:2 * kp + 2, ts(tb, 512)],
                                             start=(kp == 0), stop=(kp == 3),
                                             perf_mode=DR)
                        nc.scalar.add(ka[:, jo, ts(tb, 512)], pk[:], bk_s[:, jo:jo + 1])
                        pswk = psum("pswk")
                        nc.tensor.matmul(pswk[:], permw[:], ka[:, jo, ts(tb, 512)],
                                         start=True, stop=True)
                        s2 = scr("s2k")
                        nc.vector.tensor_tensor(s2[:], pswk[:], ss[:, ts(tb, 512)],
                                                ALU.mult)
                        s1 = scr("s1k")
                        nc.vector.tensor_tensor(s1[:], ka[:, jo, ts(tb, 512)],
                                                cc[:, ts(tb, 512)], ALU.mult)
                        nc.vector.tensor_tensor(kr[:, jo, ts(tb, 512)], s1[:],
                                                s2[:], ALU.add)

                # v, token-major, ones-column per head (denominator trick)
                v_sb = sb.tile([128, KO, H, 65], f8, tag="vsb", bufs=1)
                nc.vector.memset(v_sb[:, :, :, 64:65], 1.0)
                for nb in range(2):
                    for to in range(KO):
                        pv = psum("pv")
                        for kp in range(4):
                            nc.tensor.matmul(pv[:], g8[:, 2 * kp:2 * kp + 2, ts(to, 128)],
                                             wv8[:, 2 * kp:2 * kp + 2, ts(nb, 512)],
                                             start=(kp == 0), stop=(kp == 3),
                                             perf_mode=DR)
                        with nc.allow_low_precision(reason="fp8 v"):
                            nc.vector.tensor_copy(
                                v_sb[:, to, nb * 8:(nb + 1) * 8, 0:64],
                                pv[:].rearrange("p (h d) -> p h d", d=64))

                if stage == "proj":
                    x2e = sb.tile([128, KO, Q], f32, tag="F", bufs=2, name="x2e")
                    nc.vector.tensor_tensor(x2e[:], qr[:], kr[:, :, 0:Q], ALU.add)
                    nc.vector.tensor_tensor(x2e[:, 0, :], x2e[:, 0, :], v_sb[:, 0, 0:4, 0:64].rearrange("p a b -> p (a b)")[:, 0:512], ALU.add)
                    nc.sync.dma_start(yt_d[:], x2e[:])
                    continue
                # ---- P3: attention ----
                oT8 = sb.tile([128, KO, Q], bf, tag="B", bufs=3, name="oT8")
                for hp in range(8):
                    jo = hp
                    probs = {0: [], 1: []}
                    for half in range(4):
                        pbig = {}
                        for sub in range(2):
                            r0 = sub * 64
                            big = psum2("sc")
                            for kk in range(2):
                                kt = half * 2 + kk
                                nc.tensor.matmul(big[:, ts(kk, 512)],
                                                 kr[r0:r0 + 64, jo, ts(kt, 128)],
                                                 qr[r0:r0 + 64, jo, :],
                                                 start=True, stop=True,
                                                 tile_position=(r0, 0))
                            pbig[sub] = big
                        for sub in range(2):
                            pb = sb.tile([128, 2, 512], f8, tag="probs", bufs=4,
                                         name="pb")
                            with nc.allow_low_precision(reason="fp8 probs"):
                                nc.scalar.activation(pb[:], pbig[sub][:], AF.Exp,
                                                     scale=0.125, bias=ln2_ap[:])
                            probs[sub].append(pb)
                    for sub in range(2):
                        h = 2 * hp + sub
                        r0 = sub * 64
                        po = psum("po")
                        for hf in range(4):
                            nc.tensor.matmul(po[0:65, :],
                                             v_sb[:, 2 * hf:2 * hf + 2, h, 0:65],
                                             probs[sub][hf][:],
                                             start=(hf == 0), stop=(hf == 3),
                                             perf_mode=DR)
                        rcp = sb.tile([65, 512], bf, tag="rcp", bufs=2, name="rcp")
                        with nc.allow_low_precision(reason="bf16 softmax denom"):
                            nc.vector.reciprocal(rcp[64:65, :], po[64:65, :])
                        prb = psum("prb")
                        nc.tensor.matmul(prb[0:64, :], ones_b[64:65, 0:64],
                                         rcp[64:65, :], start=True, stop=True)
                        rb = sb.tile([64, 512], bf, tag="rb", bufs=2, name="rb")
                        nc.vector.tensor_copy(rb[:], prb[0:64, :])
                        nc.vector.tensor_tensor(oT8[r0:r0 + 64, hp, :],
                                                po[0:64, :], rb[:], ALU.mult)

                if stage == "attn":
                    x2e = sb.tile([128, KO, Q], f32, tag="F", bufs=2, name="x2e2")
                    nc.vector.tensor_copy(x2e[:], oT8[:])
                    nc.sync.dma_start(yt_d[:], x2e[:])
                    continue
                # ---- P4: attn out + gated residual ----
                x2 = sb.tile([128, KO, Q], f32, tag="F", bufs=2, name="x2")
                for do in range(KO):
                    py = psum("py")
                    for ko in range(KO):
                        nc.tensor.matmul(py[:], wao_sb[do // 4][:, ko, ts(do % 4, 128)],
                                         oT8[:, ko, :], start=(ko == 0),
                                         stop=(ko == KO - 1))
                    nc.scalar.activation(x2[:, do, :], py[:], AF.Identity,
                                         bias=bo_s[:, do:do + 1],
                                         scale=gm_s[:, do:do + 1])
                # ---- P5: LN2 (512 tokens) ----
                x2b = sb.tile([128, KO, Q], bf, tag="B", bufs=3, name="x2b")
                sq2 = sb.tile([128, KO, Q], bf, tag="B", bufs=3, name="sq2")
                for hh in range(2):
                    c = slice(4 * hh, 4 * hh + 4)
                    nc.vector.tensor_tensor(x2[:, c, :], x2[:, c, :],
                                            xskip[:, c, :], ALU.add)
                    nc.scalar.copy(x2b[:, c, :], x2[:, c, :])
                    nc.scalar.square(sq2[:, c, :], x2[:, c, :])
                p1 = psum("lp1")
                p2 = psum("lp2")
                for ko in range(KO):
                    nc.tensor.matmul(p1[:], ones_b[:], x2b[:, ko, :],
                                     start=(ko == 0), stop=(ko == KO - 1))
                    nc.tensor.matmul(p2[:], ones_b[:], sq2[:, ko, :],
                                     start=(ko == 0), stop=(ko == KO - 1))
                mu = tmpf("mu2")
                nc.vector.tensor_scalar_mul(mu[:], p1[:], 1.0 / D)
                ex2 = tmpf("ex22")
                nc.vector.tensor_scalar_mul(ex2[:], p2[:], 1.0 / D)
                var = tmpf("var2")
                nc.vector.tensor_tensor(var[:], mu[:], mu[:], ALU.mult)
                nc.vector.tensor_tensor(ex2[:], ex2[:], var[:], ALU.subtract)
                nc.scalar.activation(var[:], ex2[:], AF.Sqrt, bias=eps_ap[:])
                mu16 = sb.tile([128, 512], bf, tag="stats16", bufs=2, name="mu16")
                nc.vector.tensor_copy(mu16[:], mu[:])
                rstd16 = sb.tile([128, 512], bf, tag="stats16", bufs=2, name="rstd16")
                with nc.allow_low_precision(reason="bf16 LN rstd"):
                    nc.vector.reciprocal(rstd16[:], var[:])
                pka2 = ps.tile([128, 512], f32, tag="p", bufs=4, name="pka2")
                for _ka in range(6):
                    nc.tensor.matmul(pka2[:], ones_b[:], mu16[:],
                                     start=(_ka == 0), stop=(_ka == 5))
                mu2_b = mu16[:].unsqueeze(1).broadcast_to([128, 4, Q])
                rstd2_b = rstd16[:].unsqueeze(1).broadcast_to([128, 4, Q])
                # in-place LN2 apply: x2b <- (x2b - mu) * rstd
                for hh in range(2):
                    c = slice(4 * hh, 4 * hh + 4)
                    nc.vector.tensor_tensor(x2b[:, c, :], x2b[:, c, :], mu2_b,
                                            ALU.subtract)
                    nc.vector.tensor_tensor(x2b[:, c, :], x2b[:, c, :],
                                            rstd2_b, ALU.mult)
                g2 = x2b

                if stage == "ln2":
                    nc.sync.dma_start(yt_d[:], x2[:])
                    continue
                # ---- P6/P7: MLP (bf16) ----
                m16 = sb.tile([128, 32, Q], bf, tag="m16", bufs=1)
                for mo in range(32):
                    pm = psum("pm")
                    for ko in range(KO):
                        nc.tensor.matmul(pm[:], w1_sb[mo // 4][:, ko, ts(mo % 4, 128)],
                                         g2[:, ko, :], start=(ko == 0),
                                         stop=(ko == KO - 1))
                    nc.scalar.activation(m16[:, mo], pm[:], AF.Gelu_apprx_tanh,
                                         bias=b1_s[:, mo:mo + 1], scale=1.0)
                for do in range(KO):
                    pz = psum("pz")
                    for ko in range(32):
                        nc.tensor.matmul(pz[:], w2_sb[do][:, ko, :],
                                         m16[:, ko, :], start=(ko == 0),
                                         stop=(ko == 31))
                    t = tmpf("t")
                    nc.scalar.activation(t[:], pz[:], AF.Identity,
                                         bias=b2_s[:, do:do + 1],
                                         scale=gp_s[:, do:do + 1])
                    nc.vector.tensor_tensor(x2[:, do, :], x2[:, do, :], t[:],
                                            ALU.add)
                    if do == 3:
                        nc.sync.dma_start(yt_d[:, 0:4, :], x2[:, 0:4, :])
                nc.sync.dma_start(yt_d[:, 4:8, :], x2[:, 4:8, :])

    nc.compile()
    return nc


# ----------------------------------------------------------------------------
# host wrapper
# ----------------------------------------------------------------------------

def _pieces(W, m_piece):
    """[K, M] weight -> [n_pieces, 128, K//128, m_piece], contiguous f32."""
    K, M = W.shape
    ko = K // 128
    Wr = np.asarray(W, np.float32).reshape(ko, 128, M).transpose(1, 0, 2)
    n = M // m_piece
    out = Wr.reshape(128, ko, n, m_piece).transpose(2, 0, 1, 3)
    return np.ascontiguousarray(out)


def _fold(W):
    """[K, M] -> [128, K//128, M] (partition-major chunks of the K axis)."""
    K, M = W.shape
    return np.ascontiguousarray(
        np.asarray(W, np.float32).reshape(K // 128, 128, M).transpose(1, 0, 2))


def _pvec(v):
    v = np.asarray(v, np.float32)
    return np.ascontiguousarray(v.reshape(-1, 128).T)


def _prep_shared(inputs):
    x = np.asarray(inputs["x"], np.float32)
    c = np.asarray(inputs["c"], np.float32)
    w_ada = np.asarray(inputs["w_ada"], np.float32)
    b_ada = np.asarray(inputs["b_ada"], np.float32)
    w_qkv = np.asarray(inputs["w_qkv"], np.float32)
    w_ao = np.asarray(inputs["w_attn_out"], np.float32)
    w_m1 = np.asarray(inputs["w_mlp1"], np.float32)
    w_m2 = np.asarray(inputs["w_mlp2"], np.float32)

    mod = c @ w_ada + b_ada
    sh_msa, sc_msa, g_msa, sh_mlp, sc_mlp, g_mlp = np.split(mod, 6, axis=1)
    ln1 = np.asarray(inputs["w_ln1"], np.float32) * (1.0 + sc_msa)
    ln2 = np.asarray(inputs["w_ln2"], np.float32) * (1.0 + sc_mlp)

    shared = {}
    for b in range(B):
        Wq = w_qkv[:, :D] * ln1[b][:, None]
        Wk = w_qkv[:, D:2 * D] * ln1[b][:, None]
        Wv = w_qkv[:, 2 * D:] * ln1[b][:, None]
        bqkv = sh_msa[b] @ w_qkv
        W1 = w_m1 * ln2[b][:, None]
        bm1 = sh_mlp[b] @ w_m1 + np.asarray(inputs["b_mlp1"], np.float32)
        biases = np.concatenate([
            _pvec(bqkv[:D]), _pvec(bqkv[D:2 * D]),
            _pvec((bqkv[2 * D:] @ w_ao) * g_msa[b]), _pvec(g_msa[b]),
            _pvec(bm1),
            _pvec(np.asarray(inputs["b_mlp2"], np.float32) * g_mlp[b]),
            _pvec(g_mlp[b]),
        ], axis=1).astype(np.float32)
        shared[b] = dict(
            wq8=np.ascontiguousarray(_fold(Wq).astype(F8)),
            wk8=np.ascontiguousarray(_fold(Wk).astype(F8)),
            wv8=np.ascontiguousarray(_fold(Wv).astype(F8)),
            wm1=np.ascontiguousarray(_pieces(W1, 512).astype(BF)),
            bias=np.ascontiguousarray(biases),
        )
    wao_p = np.ascontiguousarray(_pieces(w_ao, 512).astype(BF))
    wm2_p = np.ascontiguousarray(_pieces(w_m2, 128).astype(BF))
    cos = np.asarray(inputs["cos"], np.float32)
    sin = np.asarray(inputs["sin"], np.float32)

    sw = np.arange(128)
    sw = np.where(sw % 64 < 32, sw + 32, sw - 32)
    permw = np.zeros((128, 128), np.float32)
    permw[sw, np.arange(128)] = 1.0
    permw = np.ascontiguousarray(permw.astype(BF))
    return shared, wao_p, wm2_p, permw, x, cos, sin


def _make_in_maps(inputs):
    shared, wao_p, wm2_p, permw, x, cos, sin = _prep_shared(inputs)
    in_maps = []
    for core in range(8):
        b, half = core // 2, core % 2
        qlo = half * Q
        order = np.concatenate([np.arange(qlo, qlo + Q), np.arange(0, qlo),
                                np.arange(qlo + Q, S)])
        xT = x[b][order].T                       # [D, S]
        cosT = cos[order].T                      # [32, S]
        sinT = sin[order].T
        cc = np.concatenate([cosT] * 4, 0)
        ssm = np.concatenate([-sinT, sinT, -sinT, sinT], 0)
        ccss = np.ascontiguousarray(
            np.concatenate([cc, ssm], axis=1).astype(BF))   # [128, 2S]
        xf = xT.reshape(KO, 128, S).transpose(1, 0, 2)
        sh = shared[b]
        in_maps.append({
            "xb": np.ascontiguousarray(xf.astype(BF)),
            "xskip": np.ascontiguousarray(xf[:, :, :Q].astype(np.float32)),
            "ccss": ccss,
            "bias": sh["bias"],
            "permw": permw,
            "wq8": sh["wq8"], "wk8": sh["wk8"], "wv8": sh["wv8"],
            "wao": wao_p,
            "wm1": sh["wm1"], "wm2": wm2_p,
        })
    return in_maps


def kernel(**inputs):
    from concourse import bass_utils

    if "nc" not in _CACHE:
        _CACHE["nc"] = _build_program()
    nc = _CACHE["nc"]

    in_maps = _make_in_maps(inputs)
    res = bass_utils.run_bass_kernel_spmd(nc, in_maps, core_ids=list(range(8)))

    y = np.zeros((B, S, D), np.float32)
    for core in range(8):
        b, half = core // 2, core % 2
        qlo = half * Q
        yt = res.results[core]["yt"]             # [128, KO, Q]
        y[b, qlo:qlo + Q] = yt.transpose(1, 0, 2).reshape(D, Q).T
    return y


# revision 10
# speedup vs baseline: 1.0002x; 1.0002x over previous
"""DDiT block kernel v2 for 8 Trainium2 NeuronCores.

Sharding: (batch, sequence-half) -> 8 shards, as v1. Feature-major layout
(model dim on partitions). v2 changes vs v1:
  - fp8e4m3 + DoubleRow matmuls for Q/K/V projections and attention AV
    (halves those matmul counts); scores, attn-out and MLP stay bf16.
  - rope rotate-half swap via a PE permutation matmul (no SBUF->SBUF DMA).
  - attention probs stored fp8 with a *2 exp bias (cancels in the softmax
    normalization); denominator via the ones-column trick as v1.
  - consolidated tiles and DMAs (~30 DMAs total vs 130), ring-shared SBUF
    tags with in-place LN applies to fit the 208 KiB/partition budget.
"""

import numpy as np
import ml_dtypes

BF = ml_dtypes.bfloat16
F8 = ml_dtypes.float8_e4m3

B, S, D, H, HD = 4, 1024, 1024, 16, 64
Q = 512
KO = 8
MLP = 4096
LN_EPS = 1e-5
EXPC = float(np.log(2.0))   # probs scaled by 2 (cancels in normalization)

_CACHE = {}


def _build_program(repeat=1, stage="full"):
    import concourse.bass as bass
    import concourse.mybir as mybir
    import concourse.tile as tile
    from concourse import bacc

    f32 = mybir.dt.float32
    bf = mybir.dt.bfloat16
    f8 = mybir.dt.float8e4
    AF = mybir.ActivationFunctionType
    ALU = mybir.AluOpType
    DR = mybir.MatmulPerfMode.DoubleRow
    ts = bass.ts

    nc = bacc.Bacc("TRN2", target_bir_lowering=False, debug=False,
                   enable_asserts=False)

    def din(name, shape, dt=bf):
        return nc.dram_tensor(name, shape, dt, kind="ExternalInput").ap()

    xb_d = din("xb", [128, KO, S])
    xs_d = din("xskip", [128, KO, Q], f32)
    ccss_d = din("ccss", [128, 2 * S])
    bias_d = din("bias", [128, 80], f32)
    perm_d = din("permw", [128, 128])
    wq_d = din("wq8", [128, KO, D], f8)
    wk_d = din("wk8", [128, KO, D], f8)
    wv_d = din("wv8", [128, KO, D], f8)
    wo_d = din("wao", [2, 128, KO, 512])
    w1_d = din("wm1", [8, 128, KO, 512])
    w2_d = din("wm2", [8, 128, 32, 128])
    yt_d = nc.dram_tensor("yt", [128, KO, Q], f32, kind="ExternalOutput").ap()

    with tile.TileContext(nc) as tc:
        with tc.tile_pool(name="sb", bufs=1) as sb, \
             tc.tile_pool(name="ps", bufs=1, space="PSUM") as ps:
            for _rep in range(repeat):

                def psum(name="pt"):
                    return ps.tile([128, 512], f32, tag="p", bufs=4, name=name)

                def psum2(name="pt2"):
                    return ps.tile([128, 1024], f32, tag="p2", bufs=2, name=name)

                def tmpf(name="tf"):
                    return sb.tile([128, 512], f32, tag="tmpf", bufs=3, name=name)

                def scr(name="sc"):
                    return sb.tile([128, 512], bf, tag="scr", bufs=6, name=name)

                def wall(name="w"):
                    return sb.tile([128, KO, 512], bf, tag="wall", bufs=5,
                                   name=name)

                # ---- input DMAs (weights streamed through the "wall" ring) ----
                xb = sb.tile([128, KO, S], bf, tag="A", bufs=2, name="xb")
                for _c in range(4):
                    nc.sync.dma_start(xb[:, 2 * _c:2 * _c + 2, :],
                                      xb_d[:, 2 * _c:2 * _c + 2, :])
                xskip = sb.tile([128, KO, Q], f32, tag="F", bufs=2, name="xskip")
                nc.sync.dma_start(xskip[:], xs_d[:])
                ccss = sb.tile([128, 2 * S], bf, tag="ccss", bufs=1)
                nc.sync.dma_start(ccss[:], ccss_d[:])
                cc = ccss[:, 0:S]
                ss = ccss[:, S:2 * S]
                bias = sb.tile([128, 80], f32, tag="bias", bufs=1)
                nc.sync.dma_start(bias[:], bias_d[:])
                bq_s, bk_s = bias[:, 0:8], bias[:, 8:16]
                bo_s, gm_s = bias[:, 16:24], bias[:, 24:32]
                b1_s = bias[:, 32:64]
                b2_s, gp_s = bias[:, 64:72], bias[:, 72:80]
                permw = sb.tile([128, 128], bf, tag="permw", bufs=1)
                nc.sync.dma_start(permw[:], perm_d[:])
                wq8 = sb.tile([128, KO, D], f8, tag="wall", bufs=5, name="wq8")
                nc.sync.dma_start(wq8[:], wq_d[:])
                wk8 = sb.tile([128, KO, D], f8, tag="wall", bufs=5, name="wk8")
                nc.sync.dma_start(wk8[:], wk_d[:])
                wv8 = sb.tile([128, KO, D], f8, tag="wall", bufs=5, name="wv8")
                nc.sync.dma_start(wv8[:], wv_d[:])
                wao_sb = []
                for i in range(2):
                    t = wall(f"wao{i}")
                    nc.sync.dma_start(t[:], wo_d[i])
                    wao_sb.append(t)
                w1_sb = []
                for i in range(8):
                    t = wall(f"w1p{i}")
                    nc.sync.dma_start(t[:], w1_d[i])
                    w1_sb.append(t)
                w2_sb = []
                for i in range(8):
                    t = sb.tile([128, 32, 128], bf, tag="wall", bufs=5,
                                name=f"w2p{i}")
                    nc.sync.dma_start(t[:], w2_d[i])
                    w2_sb.append(t)

                ones_b = sb.tile([128, 128], bf, tag="ones", bufs=1)
                nc.vector.memset(ones_b[:], 1.0)
                # PE warmup: ~7us of dummy matmuls while input DMAs land, so
                # the HAM clock-gate reaches 2.4 GHz before real work arrives
                pwu = ps.tile([128, 512], f32, tag="p", bufs=4, name="pwu")
                for _wu in range(16):
                    nc.tensor.matmul(pwu[:], ones_b[:], ones_b[:, 0:1].broadcast_to([128, 512]),
                                     start=(_wu == 0), stop=(_wu == 15))
                eps_ap = sb.tile([128, 1], f32, tag="eps", bufs=1)
                nc.vector.memset(eps_ap[:], LN_EPS)
                ln2_ap = sb.tile([128, 1], f32, tag="ln2c", bufs=1)
                nc.vector.memset(ln2_ap[:], EXPC)

                # ---- P1: LN1 over all 1024 tokens ----
                sq = sb.tile([128, KO, S], bf, tag="A", bufs=2, name="sq")
                for _c in range(4):
                    nc.scalar.square(sq[:, 2 * _c:2 * _c + 2, :],
                                     xb[:, 2 * _c:2 * _c + 2, :])
                ps_s1 = [psum("ps1") for _ in range(2)]
                ps_s2 = [psum("ps2") for _ in range(2)]
                for ko in range(KO):
                    for tb in range(2):
                        nc.tensor.matmul(ps_s1[tb][:], ones_b[:],
                                         xb[:, ko, ts(tb, 512)],
                                         start=(ko == 0), stop=(ko == KO - 1))
                        nc.tensor.matmul(ps_s2[tb][:], ones_b[:],
                                         sq[:, ko, ts(tb, 512)],
                                         start=(ko == 0), stop=(ko == KO - 1))

                mu01 = sb.tile([128, S], bf, tag="stats16", bufs=2, name="mu01")
                rstd01 = sb.tile([128, S], bf, tag="stats16", bufs=2, name="rstd01")
                for tb in range(2):
                    mu = tmpf("mu")
                    nc.vector.tensor_scalar_mul(mu[:], ps_s1[tb][:], 1.0 / D)
                    ex2 = tmpf("ex2")
                    nc.vector.tensor_scalar_mul(ex2[:], ps_s2[tb][:], 1.0 / D)
                    var = tmpf("var")
                    nc.vector.tensor_tensor(var[:], mu[:], mu[:], ALU.mult)
                    nc.vector.tensor_tensor(ex2[:], ex2[:], var[:], ALU.subtract)
                    nc.scalar.activation(var[:], ex2[:], AF.Sqrt, bias=eps_ap[:])
                    nc.vector.tensor_copy(mu01[:, ts(tb, 512)], mu[:])
                    with nc.allow_low_precision(reason="bf16 LN rstd"):
                        nc.vector.reciprocal(rstd01[:, ts(tb, 512)], var[:])

                pka = ps.tile([128, 512], f32, tag="p", bufs=4, name="pka")
                for _ka in range(6):
                    nc.tensor.matmul(pka[:], ones_b[:], mu01[:, 0:512],
                                     start=(_ka == 0), stop=(_ka == 5))
                mu_b = mu01[:].unsqueeze(1).broadcast_to([128, 2, S])
                rstd_b = rstd01[:].unsqueeze(1).broadcast_to([128, 2, S])
                # in-place per 2-ko chunk: xb <- xb - mu ; g8 = fp8(xb * rstd)
                g8 = sb.tile([128, KO, S], f8, tag="g8", bufs=1)
                for kp in range(4):
                    c = slice(2 * kp, 2 * kp + 2)
                    nc.vector.tensor_tensor(xb[:, c, :], xb[:, c, :], mu_b,
                                            ALU.subtract)
                    with nc.allow_low_precision(reason="fp8 LN1 activations"):
                        nc.vector.tensor_tensor(g8[:, c, :], xb[:, c, :],
                                                rstd_b, ALU.mult)

                # ---- P2: projections + rope (fp8 DoubleRow) ----
                qa = sb.tile([128, KO, Q], bf, tag="B", bufs=3, name="qa")
                qr = sb.tile([128, KO, Q], bf, tag="B", bufs=3, name="qr")
                for jo in range(KO):
                    pq = psum("pq")
                    for kp in range(4):
                        nc.tensor.matmul(pq[:], wq8[:, 2 * kp:2 * kp + 2, ts(jo, 128)],
                                         g8[:, 2 * kp:2 * kp + 2, 0:Q],
                                         start=(kp == 0), stop=(kp == 3),
                                         perf_mode=DR)
                    nc.scalar.add(qa[:, jo, :], pq[:], bq_s[:, jo:jo + 1])
                    psw = psum("psw")
                    nc.tensor.matmul(psw[:], permw[:], qa[:, jo, :],
                                     start=True, stop=True)
                    s2 = scr("s2q")
                    nc.vector.tensor_tensor(s2[:], psw[:], ss[:, 0:Q], ALU.mult)
                    s1 = scr("s1q")
                    nc.vector.tensor_tensor(s1[:], qa[:, jo, :], cc[:, 0:Q],
                                            ALU.mult)
                    nc.vector.tensor_tensor(qr[:, jo, :], s1[:], s2[:], ALU.add)

                ka = sb.tile([128, KO, S], bf, tag="A", bufs=2, name="ka")
                kr = sb.tile([128, KO, S], bf, tag="A", bufs=2, name="kr")
                for jo in range(KO):
                    for tb in range(2):
                        pk = psum("pk")
                        for kp in range(4):
                            nc.tensor.matmul(pk[:], wk8[:, 2 * kp:2 * kp + 2, ts(jo, 128)],
                                             g8[:, 2 * kp:2 * kp + 2, ts(tb, 512)],
                                             start=(kp == 0), stop=(kp == 3),
                                             perf_mode=DR)
                        nc.scalar.add(ka[:, jo, ts(tb, 512)], pk[:], bk_s[:, jo:jo + 1])
                        pswk = psum("pswk")
                        nc.tensor.matmul(pswk[:], permw[:], ka[:, jo, ts(tb, 512)],
                                         start=True, stop=True)
                        s2 = scr("s2k")
                        nc.vector.tensor_tensor(s2[:], pswk[:], ss[:, ts(tb, 512)],
                                                ALU.mult)
                        s1 = scr("s1k")
                        nc.vector.tensor_tensor(s1[:], ka[:, jo, ts(tb, 512)],
                                                cc[:, ts(tb, 512)], ALU.mult)
                        nc.vector.tensor_tensor(kr[:, jo, ts(tb, 512)], s1[:],
                                                s2[:], ALU.add)

                # v, token-major, ones-column per head (denominator trick)
                v_sb = sb.tile([128, KO, H, 65], f8, tag="vsb", bufs=1)
                nc.vector.memset(v_sb[:, :, :, 64:65], 1.0)
                for nb in range(2):
                    for to in range(KO):
                        pv = psum("pv")
                        for kp in range(4):
                            nc.tensor.matmul(pv[:], g8[:, 2 * kp:2 * kp + 2, ts(to, 128)],
                                             wv8[:, 2 * kp:2 * kp + 2, ts(nb, 512)],
                                             start=(kp == 0), stop=(kp == 3),
                                             perf_mode=DR)
                        with nc.allow_low_precision(reason="fp8 v"):
                            nc.scalar.copy(
                                v_sb[:, to, nb * 8:(nb + 1) * 8, 0:64],
                                pv[:].rearrange("p (h d) -> p h d", d=64))

                if stage == "proj":
                    x2e = sb.tile([128, KO, Q], f32, tag="F", bufs=2, name="x2e")
                    nc.vector.tensor_tensor(x2e[:], qr[:], kr[:, :, 0:Q], ALU.add)
                    nc.vector.tensor_tensor(x2e[:, 0, :], x2e[:, 0, :], v_sb[:, 0, 0:4, 0:64].rearrange("p a b -> p (a b)")[:, 0:512], ALU.add)
                    nc.sync.dma_start(yt_d[:], x2e[:])
                    continue
                # ---- P3: attention ----
                oT8 = sb.tile([128, KO, Q], bf, tag="B", bufs=3, name="oT8")
                for hp in range(8):
                    jo = hp
                    probs = {0: [], 1: []}
                    for half in range(4):
                        pbig = {}
                        for sub in range(2):
                            r0 = sub * 64
                            big = psum2("sc")
                            for kk in range(2):
                                kt = half * 2 + kk
                                nc.tensor.matmul(big[:, ts(kk, 512)],
                                                 kr[r0:r0 + 64, jo, ts(kt, 128)],
                                                 qr[r0:r0 + 64, jo, :],
                                                 start=True, stop=True,
                                                 tile_position=(r0, 0))
                            pbig[sub] = big
                        for sub in range(2):
                            pb = sb.tile([128, 2, 512], f8, tag="probs", bufs=4,
                                         name="pb")
                            with nc.allow_low_precision(reason="fp8 probs"):
                                nc.scalar.activation(pb[:], pbig[sub][:], AF.Exp,
                                                     scale=0.125, bias=ln2_ap[:])
                            probs[sub].append(pb)
                    for sub in range(2):
                        h = 2 * hp + sub
                        r0 = sub * 64
                        po = psum("po")
                        for hf in range(4):
                            nc.tensor.matmul(po[0:65, :],
                                             v_sb[:, 2 * hf:2 * hf + 2, h, 0:65],
                                             probs[sub][hf][:],
                                             start=(hf == 0), stop=(hf == 3),
                                             perf_mode=DR)
                        rcp = sb.tile([65, 512], bf, tag="rcp", bufs=2, name="rcp")
                        with nc.allow_low_precision(reason="bf16 softmax denom"):
                            nc.vector.reciprocal(rcp[64:65, :], po[64:65, :])
                        prb = psum("prb")
                        nc.tensor.matmul(prb[0:64, :], ones_b[64:65, 0:64],
                                         rcp[64:65, :], start=True, stop=True)
                        rb = sb.tile([64, 512], bf, tag="rb", bufs=2, name="rb")
                        nc.vector.tensor_copy(rb[:], prb[0:64, :])
                        nc.vector.tensor_tensor(oT8[r0:r0 + 64, hp, :],
                                                po[0:64, :], rb[:], ALU.mult)

                if stage == "attn":
                    x2e = sb.tile([128, KO, Q], f32, tag="F", bufs=2, name="x2e2")
                    nc.vector.tensor_copy(x2e[:], oT8[:])
                    nc.sync.dma_start(yt_d[:], x2e[:])
                    continue
                # ---- P4: attn out + gated residual ----
                x2 = sb.tile([128, KO, Q], f32, tag="F", bufs=2, name="x2")
                for do in range(KO):
                    py = psum("py")
                    for ko in range(KO):
                        nc.tensor.matmul(py[:], wao_sb[do // 4][:, ko, ts(do % 4, 128)],
                                         oT8[:, ko, :], start=(ko == 0),
                                         stop=(ko == KO - 1))
                    nc.scalar.activation(x2[:, do, :], py[:], AF.Identity,
                                         bias=bo_s[:, do:do + 1],
                                         scale=gm_s[:, do:do + 1])
                # ---- P5: LN2 (512 tokens) ----
                x2b = sb.tile([128, KO, Q], bf, tag="B", bufs=3, name="x2b")
                sq2 = sb.tile([128, KO, Q], bf, tag="B", bufs=3, name="sq2")
                for hh in range(2):
                    c = slice(4 * hh, 4 * hh + 4)
                    nc.vector.tensor_tensor(x2[:, c, :], x2[:, c, :],
                                            xskip[:, c, :], ALU.add)
                    nc.scalar.copy(x2b[:, c, :], x2[:, c, :])
                    nc.scalar.square(sq2[:, c, :], x2[:, c, :])
                p1 = psum("lp1")
                p2 = psum("lp2")
                for ko in range(KO):
                    nc.tensor.matmul(p1[:], ones_b[:], x2b[:, ko, :],
                                     start=(ko == 0), stop=(ko == KO - 1))
                    nc.tensor.matmul(p2[:], ones_b[:], sq2[:, ko, :],
                                     start=(ko == 0), stop=(ko == KO - 1))
                mu = tmpf("mu2")
                nc.vector.tensor_scalar_mul(mu[:], p1[:], 1.0 / D)
                ex2 = tmpf("ex22")
                nc.vector.tensor_scalar_mul(ex2[:], p2[:], 1.0 / D)
                var = tmpf("var2")
                nc.vector.tensor_tensor(var[:], mu[:], mu[:], ALU.mult)
                nc.vector.tensor_tensor(ex2[:], ex2[:], var[:], ALU.subtract)
                nc.scalar.activation(var[:], ex2[:], AF.Sqrt, bias=eps_ap[:])
                mu16 = sb.tile([128, 512], bf, tag="stats16", bufs=2, name="mu16")
                nc.vector.tensor_copy(mu16[:], mu[:])
                rstd16 = sb.tile([128, 512], bf, tag="stats16", bufs=2, name="rstd16")
                with nc.allow_low_precision(reason="bf16 LN rstd"):
                    nc.vector.reciprocal(rstd16[:], var[:])
                pka2 = ps.tile([128, 512], f32, tag="p", bufs=4, name="pka2")
                for _ka in range(6):
                    nc.tensor.matmul(pka2[:], ones_b[:], mu16[:],
                                     start=(_ka == 0), stop=(_ka == 5))
                mu2_b = mu16[:].unsqueeze(1).broadcast_to([128, 4, Q])
                rstd2_b = rstd16[:].unsqueeze(1).broadcast_to([128, 4, Q])
                # in-place LN2 apply: x2b <- (x2b - mu) * rstd
                for hh in range(2):
                    c = slice(4 * hh, 4 * hh + 4)
                    nc.vector.tensor_tensor(x2b[:, c, :], x2b[:, c, :], mu2_b,
                                            ALU.subtract)
                    nc.vector.tensor_tensor(x2b[:, c, :], x2b[:, c, :],
                                            rstd2_b, ALU.mult)
                g2 = x2b

                if stage == "ln2":
                    nc.sync.dma_start(yt_d[:], x2[:])
                    continue
                # ---- P6/P7: MLP (bf16) ----
                m16 = sb.tile([128, 32, Q], bf, tag="m16", bufs=1)
                for mo in range(32):
                    pm = psum("pm")
                    for ko in range(KO):
                        nc.tensor.matmul(pm[:], w1_sb[mo // 4][:, ko, ts(mo % 4, 128)],
                                         g2[:, ko, :], start=(ko == 0),
                                         stop=(ko == KO - 1))
                    nc.scalar.activation(m16[:, mo], pm[:], AF.Gelu_apprx_tanh,
                                         bias=b1_s[:, mo:mo + 1], scale=1.0)
                for do in range(KO):
                    pz = psum("pz")
                    for ko in range(32):
                        nc.tensor.matmul(pz[:], w2_sb[do][:, ko, :],
                                         m16[:, ko, :], start=(ko == 0),
                                         stop=(ko == 31))
                    t = tmpf("t")
                    nc.scalar.activation(t[:], pz[:], AF.Identity,
                                         bias=b2_s[:, do:do + 1],
                                         scale=gp_s[:, do:do + 1])
                    nc.vector.tensor_tensor(x2[:, do, :], x2[:, do, :], t[:],
                                            ALU.add)
                    if do == 3:
                        nc.sync.dma_start(yt_d[:, 0:4, :], x2[:, 0:4, :])
                nc.sync.dma_start(yt_d[:, 4:8, :], x2[:, 4:8, :])

    nc.compile()
    return nc


# ----------------------------------------------------------------------------
# host wrapper
# ----------------------------------------------------------------------------

def _pieces(W, m_piece):
    """[K, M] weight -> [n_pieces, 128, K//128, m_piece], contiguous f32."""
    K, M = W.shape
    ko = K // 128
    Wr = np.asarray(W, np.float32).reshape(ko, 128, M).transpose(1, 0, 2)
    n = M // m_piece
    out = Wr.reshape(128, ko, n, m_piece).transpose(2, 0, 1, 3)
    return np.ascontiguousarray(out)


def _fold(W):
    """[K, M] -> [128, K//128, M] (partition-major chunks of the K axis)."""
    K, M = W.shape
    return np.ascontiguousarray(
        np.asarray(W, np.float32).reshape(K // 128, 128, M).transpose(1, 0, 2))


def _pvec(v):
    v = np.asarray(v, np.float32)
    return np.ascontiguousarray(v.reshape(-1, 128).T)


def _prep_shared(inputs):
    x = np.asarray(inputs["x"], np.float32)
    c = np.asarray(inputs["c"], np.float32)
    w_ada = np.asarray(inputs["w_ada"], np.float32)
    b_ada = np.asarray(inputs["b_ada"], np.float32)
    w_qkv = np.asarray(inputs["w_qkv"], np.float32)
    w_ao = np.asarray(inputs["w_attn_out"], np.float32)
    w_m1 = np.asarray(inputs["w_mlp1"], np.float32)
    w_m2 = np.asarray(inputs["w_mlp2"], np.float32)

    mod = c @ w_ada + b_ada
    sh_msa, sc_msa, g_msa, sh_mlp, sc_mlp, g_mlp = np.split(mod, 6, axis=1)
    ln1 = np.asarray(inputs["w_ln1"], np.float32) * (1.0 + sc_msa)
    ln2 = np.asarray(inputs["w_ln2"], np.float32) * (1.0 + sc_mlp)

    shared = {}
    for b in range(B):
        Wq = w_qkv[:, :D] * ln1[b][:, None]
        Wk = w_qkv[:, D:2 * D] * ln1[b][:, None]
        Wv = w_qkv[:, 2 * D:] * ln1[b][:, None]
        bqkv = sh_msa[b] @ w_qkv
        W1 = w_m1 * ln2[b][:, None]
        bm1 = sh_mlp[b] @ w_m1 + np.asarray(inputs["b_mlp1"], np.float32)
        biases = np.concatenate([
            _pvec(bqkv[:D]), _pvec(bqkv[D:2 * D]),
            _pvec((bqkv[2 * D:] @ w_ao) * g_msa[b]), _pvec(g_msa[b]),
            _pvec(bm1),
            _pvec(np.asarray(inputs["b_mlp2"], np.float32) * g_mlp[b]),
            _pvec(g_mlp[b]),
        ], axis=1).astype(np.float32)
        shared[b] = dict(
            wq8=np.ascontiguousarray(_fold(Wq).astype(F8)),
            wk8=np.ascontiguousarray(_fold(Wk).astype(F8)),
            wv8=np.ascontiguousarray(_fold(Wv).astype(F8)),
            wm1=np.ascontiguousarray(_pieces(W1, 512).astype(BF)),
            bias=np.ascontiguousarray(biases),
        )
    wao_p = np.ascontiguousarray(_pieces(w_ao, 512).astype(BF))
    wm2_p = np.ascontiguousarray(_pieces(w_m2, 128).astype(BF))
    cos = np.asarray(inputs["cos"], np.float32)
    sin = np.asarray(inputs["sin"], np.float32)

    sw = np.arange(128)
    sw = np.where(sw % 64 < 32, sw + 32, sw - 32)
    permw = np.zeros((128, 128), np.float32)
    permw[sw, np.arange(128)] = 1.0
    permw = np.ascontiguousarray(permw.astype(BF))
    return shared, wao_p, wm2_p, permw, x, cos, sin


def _make_in_maps(inputs):
    shared, wao_p, wm2_p, permw, x, cos, sin = _prep_shared(inputs)
    in_maps = []
    for core in range(8):
        b, half = core // 2, core % 2
        qlo = half * Q
        order = np.concatenate([np.arange(qlo, qlo + Q), np.arange(0, qlo),
                                np.arange(qlo + Q, S)])
        xT = x[b][order].T                       # [D, S]
        cosT = cos[order].T                      # [32, S]
        sinT = sin[order].T
        cc = np.concatenate([cosT] * 4, 0)
        ssm = np.concatenate([-sinT, sinT, -sinT, sinT], 0)
        ccss = np.ascontiguousarray(
            np.concatenate([cc, ssm], axis=1).astype(BF))   # [128, 2S]
        xf = xT.reshape(KO, 128, S).transpose(1, 0, 2)
        sh = shared[b]
        in_maps.append({
            "xb": np.ascontiguousarray(xf.astype(BF)),
            "xskip": np.ascontiguousarray(xf[:, :, :Q].astype(np.float32)),
            "ccss": ccss,
            "bias": sh["bias"],
            "permw": permw,
            "wq8": sh["wq8"], "wk8": sh["wk8"], "wv8": sh["wv8"],
            "wao": wao_p,
            "wm1": sh["wm1"], "wm2": wm2_p,
        })
    return in_maps


def kernel(**inputs):
    from concourse import bass_utils

    if "nc" not in _CACHE:
        _CACHE["nc"] = _build_program()
    nc = _CACHE["nc"]

    in_maps = _make_in_maps(inputs)
    res = bass_utils.run_bass_kernel_spmd(nc, in_maps, core_ids=list(range(8)))

    y = np.zeros((B, S, D), np.float32)
    for core in range(8):
        b, half = core // 2, core % 2
        qlo = half * Q
        yt = res.results[core]["yt"]             # [128, KO, Q]
        y[b, qlo:qlo + Q] = yt.transpose(1, 0, 2).reshape(D, Q).T
    return y


# revision 11
# speedup vs baseline: 1.0252x; 1.0250x over previous
"""DDiT block kernel v2 for 8 Trainium2 NeuronCores.

Sharding: (batch, sequence-half) -> 8 shards, as v1. Feature-major layout
(model dim on partitions). v2 changes vs v1:
  - fp8e4m3 + DoubleRow matmuls for Q/K/V projections and attention AV
    (halves those matmul counts); scores, attn-out and MLP stay bf16.
  - rope rotate-half swap via a PE permutation matmul (no SBUF->SBUF DMA).
  - attention probs stored fp8 with a *2 exp bias (cancels in the softmax
    normalization); denominator via the ones-column trick as v1.
  - consolidated tiles and DMAs (~30 DMAs total vs 130), ring-shared SBUF
    tags with in-place LN applies to fit the 208 KiB/partition budget.
"""

import numpy as np
import ml_dtypes

BF = ml_dtypes.bfloat16
F8 = ml_dtypes.float8_e4m3

B, S, D, H, HD = 4, 1024, 1024, 16, 64
Q = 512
KO = 8
MLP = 4096
LN_EPS = 1e-5
EXPC = float(np.log(2.0))   # probs scaled by 2 (cancels in normalization)

_CACHE = {}


def _build_program(repeat=1, stage="full"):
    import concourse.bass as bass
    import concourse.mybir as mybir
    import concourse.tile as tile
    from concourse import bacc

    f32 = mybir.dt.float32
    bf = mybir.dt.bfloat16
    f8 = mybir.dt.float8e4
    AF = mybir.ActivationFunctionType
    ALU = mybir.AluOpType
    DR = mybir.MatmulPerfMode.DoubleRow
    ts = bass.ts

    nc = bacc.Bacc("TRN2", target_bir_lowering=False, debug=False,
                   enable_asserts=False)

    def din(name, shape, dt=bf):
        return nc.dram_tensor(name, shape, dt, kind="ExternalInput").ap()

    xb_d = din("xb", [128, KO, S])
    xs_d = din("xskip", [128, KO, Q], f32)
    ccss_d = din("ccss", [128, 2 * S])
    bias_d = din("bias", [128, 80], f32)
    perm_d = din("permw", [128, 128])
    wq_d = din("wq8", [128, KO, D], f8)
    wk_d = din("wk8", [128, KO, D], f8)
    wv_d = din("wv8", [128, KO, D], f8)
    wo_d = din("wao", [2, 128, KO, 512])
    w1_d = din("wm1", [8, 128, KO, 512])
    w2_d = din("wm2", [8, 128, 32, 128])
    yt_d = nc.dram_tensor("yt", [128, KO, Q], f32, kind="ExternalOutput").ap()

    with tile.TileContext(nc) as tc:
        with tc.tile_pool(name="sb", bufs=1) as sb, \
             tc.tile_pool(name="ps", bufs=1, space="PSUM") as ps:
            for _rep in range(repeat):

                def psum(name="pt"):
                    return ps.tile([128, 512], f32, tag="p", bufs=4, name=name)

                def psum2(name="pt2"):
                    return ps.tile([128, 1024], f32, tag="p2", bufs=2, name=name)

                def tmpf(name="tf"):
                    return sb.tile([128, 512], f32, tag="tmpf", bufs=4, name=name)

                def scr(name="sc"):
                    return sb.tile([128, 512], bf, tag="scr", bufs=6, name=name)

                def wall(name="w"):
                    return sb.tile([128, KO, 512], bf, tag="wall", bufs=5,
                                   name=name)

                # ---- input DMAs (weights streamed through the "wall" ring) ----
                xb = sb.tile([128, KO, S], bf, tag="A", bufs=2, name="xb")
                for _c in range(4):
                    nc.sync.dma_start(xb[:, 2 * _c:2 * _c + 2, :],
                                      xb_d[:, 2 * _c:2 * _c + 2, :])
                xskip = sb.tile([128, KO, Q], f32, tag="F", bufs=2, name="xskip")
                nc.sync.dma_start(xskip[:], xs_d[:])
                ccss = sb.tile([128, 2 * S], bf, tag="ccss", bufs=1)
                nc.sync.dma_start(ccss[:], ccss_d[:])
                cc = ccss[:, 0:S]
                ss = ccss[:, S:2 * S]
                bias = sb.tile([128, 80], f32, tag="bias", bufs=1)
                nc.sync.dma_start(bias[:], bias_d[:])
                bq_s, bk_s = bias[:, 0:8], bias[:, 8:16]
                bo_s, gm_s = bias[:, 16:24], bias[:, 24:32]
                b1_s = bias[:, 32:64]
                b2_s, gp_s = bias[:, 64:72], bias[:, 72:80]
                permw = sb.tile([128, 128], bf, tag="permw", bufs=1)
                nc.sync.dma_start(permw[:], perm_d[:])
                wq8 = sb.tile([128, KO, D], f8, tag="wall", bufs=5, name="wq8")
                nc.sync.dma_start(wq8[:], wq_d[:])
                wk8 = sb.tile([128, KO, D], f8, tag="wall", bufs=5, name="wk8")
                nc.sync.dma_start(wk8[:], wk_d[:])
                wv8 = sb.tile([128, KO, D], f8, tag="wall", bufs=5, name="wv8")
                nc.sync.dma_start(wv8[:], wv_d[:])
                wao_sb = []
                for i in range(2):
                    t = wall(f"wao{i}")
                    nc.sync.dma_start(t[:], wo_d[i])
                    wao_sb.append(t)
                w1_sb = []
                for i in range(8):
                    t = wall(f"w1p{i}")
                    nc.sync.dma_start(t[:], w1_d[i])
                    w1_sb.append(t)
                w2_sb = []
                for i in range(8):
                    t = sb.tile([128, 32, 128], bf, tag="wall", bufs=5,
                                name=f"w2p{i}")
                    nc.sync.dma_start(t[:], w2_d[i])
                    w2_sb.append(t)

                ones_b = sb.tile([128, 128], bf, tag="ones", bufs=1)
                nc.vector.memset(ones_b[:], 1.0)
                # PE warmup: ~7us of dummy matmuls while input DMAs land, so
                # the HAM clock-gate reaches 2.4 GHz before real work arrives
                pwu = ps.tile([128, 512], f32, tag="p", bufs=4, name="pwu")
                for _wu in range(16):
                    nc.tensor.matmul(pwu[:], ones_b[:], ones_b[:, 0:1].broadcast_to([128, 512]),
                                     start=(_wu == 0), stop=(_wu == 15))
                eps_ap = sb.tile([128, 1], f32, tag="eps", bufs=1)
                nc.vector.memset(eps_ap[:], LN_EPS)
                ln2_ap = sb.tile([128, 1], f32, tag="ln2c", bufs=1)
                nc.vector.memset(ln2_ap[:], EXPC)

                # ---- P1: LN1 over all 1024 tokens ----
                sq = sb.tile([128, KO, S], bf, tag="A", bufs=2, name="sq")
                for _c in range(4):
                    nc.scalar.square(sq[:, 2 * _c:2 * _c + 2, :],
                                     xb[:, 2 * _c:2 * _c + 2, :])
                ps_s1 = [psum("ps1") for _ in range(2)]
                ps_s2 = [psum("ps2") for _ in range(2)]
                for ko in range(KO):
                    for tb in range(2):
                        nc.tensor.matmul(ps_s1[tb][:], ones_b[:],
                                         xb[:, ko, ts(tb, 512)],
                                         start=(ko == 0), stop=(ko == KO - 1))
                        nc.tensor.matmul(ps_s2[tb][:], ones_b[:],
                                         sq[:, ko, ts(tb, 512)],
                                         start=(ko == 0), stop=(ko == KO - 1))

                mu01 = sb.tile([128, S], bf, tag="stats16", bufs=2, name="mu01")
                rstd01 = sb.tile([128, S], bf, tag="stats16", bufs=2, name="rstd01")
                for tb in range(2):
                    mu = tmpf("mu")
                    nc.vector.tensor_scalar_mul(mu[:], ps_s1[tb][:], 1.0 / D)
                    ex2 = tmpf("ex2")
                    nc.vector.tensor_scalar_mul(ex2[:], ps_s2[tb][:], 1.0 / D)
                    var = tmpf("var")
                    nc.vector.tensor_tensor(var[:], mu[:], mu[:], ALU.mult)
                    nc.vector.tensor_tensor(ex2[:], ex2[:], var[:], ALU.subtract)
                    nc.scalar.activation(var[:], ex2[:], AF.Sqrt, bias=eps_ap[:])
                    nc.vector.tensor_copy(mu01[:, ts(tb, 512)], mu[:])
                    with nc.allow_low_precision(reason="bf16 LN rstd"):
                        nc.vector.reciprocal(rstd01[:, ts(tb, 512)], var[:])

                pka = ps.tile([128, 512], f32, tag="p", bufs=4, name="pka")
                for _ka in range(6):
                    nc.tensor.matmul(pka[:], ones_b[:], mu01[:, 0:512],
                                     start=(_ka == 0), stop=(_ka == 5))
                # in-place per (2-ko chunk, token-half): xb <- xb - mu ;
                # g8 = fp8(xb * rstd). Token-half split so Q projection
                # (tokens 0:512) unlocks after the tb=0 stats chain alone.
                g8 = sb.tile([128, KO, S], f8, tag="g8", bufs=1)
                for tb in range(2):
                    tsl = ts(tb, 512)
                    mu_b = mu01[:, tsl].unsqueeze(1).broadcast_to([128, 2, 512])
                    rstd_b = rstd01[:, tsl].unsqueeze(1).broadcast_to([128, 2, 512])
                    for kp in range(4):
                        c = slice(2 * kp, 2 * kp + 2)
                        nc.vector.tensor_tensor(xb[:, c, tsl], xb[:, c, tsl],
                                                mu_b, ALU.subtract)
                        with nc.allow_low_precision(reason="fp8 LN1 activations"):
                            nc.vector.tensor_tensor(g8[:, c, tsl], xb[:, c, tsl],
                                                    rstd_b, ALU.mult)

                # ---- P2: projections + rope (fp8 DoubleRow) ----
                qa = sb.tile([128, KO, Q], bf, tag="B", bufs=3, name="qa")
                qr = sb.tile([128, KO, Q], bf, tag="B", bufs=3, name="qr")
                for jo in range(KO):
                    pq = psum("pq")
                    for kp in range(4):
                        nc.tensor.matmul(pq[:], wq8[:, 2 * kp:2 * kp + 2, ts(jo, 128)],
                                         g8[:, 2 * kp:2 * kp + 2, 0:Q],
                                         start=(kp == 0), stop=(kp == 3),
                                         perf_mode=DR)
                    nc.scalar.add(qa[:, jo, :], pq[:], bq_s[:, jo:jo + 1])
                    psw = psum("psw")
                    nc.tensor.matmul(psw[:], permw[:], qa[:, jo, :],
                                     start=True, stop=True)
                    s2 = scr("s2q")
                    nc.vector.tensor_tensor(s2[:], psw[:], ss[:, 0:Q], ALU.mult)
                    s1 = scr("s1q")
                    nc.vector.tensor_tensor(s1[:], qa[:, jo, :], cc[:, 0:Q],
                                            ALU.mult)
                    nc.vector.tensor_tensor(qr[:, jo, :], s1[:], s2[:], ALU.add)

                ka = sb.tile([128, KO, S], bf, tag="A", bufs=2, name="ka")
                kr = sb.tile([128, KO, S], bf, tag="A", bufs=2, name="kr")
                for jo in range(KO):
                    for tb in range(2):
                        pk = psum("pk")
                        for kp in range(4):
                            nc.tensor.matmul(pk[:], wk8[:, 2 * kp:2 * kp + 2, ts(jo, 128)],
                                             g8[:, 2 * kp:2 * kp + 2, ts(tb, 512)],
                                             start=(kp == 0), stop=(kp == 3),
                                             perf_mode=DR)
                        nc.scalar.add(ka[:, jo, ts(tb, 512)], pk[:], bk_s[:, jo:jo + 1])
                        pswk = psum("pswk")
                        nc.tensor.matmul(pswk[:], permw[:], ka[:, jo, ts(tb, 512)],
                                         start=True, stop=True)
                        s2 = scr("s2k")
                        nc.vector.tensor_tensor(s2[:], pswk[:], ss[:, ts(tb, 512)],
                                                ALU.mult)
                        s1 = scr("s1k")
                        nc.vector.tensor_tensor(s1[:], ka[:, jo, ts(tb, 512)],
                                                cc[:, ts(tb, 512)], ALU.mult)
                        nc.vector.tensor_tensor(kr[:, jo, ts(tb, 512)], s1[:],
                                                s2[:], ALU.add)

                # v, token-major, ones-column per head (denominator trick)
                v_sb = sb.tile([128, KO, H, 65], f8, tag="vsb", bufs=1)
                nc.vector.memset(v_sb[:, :, :, 64:65], 1.0)
                for nb in range(2):
                    for to in range(KO):
                        pv = psum("pv")
                        for kp in range(4):
                            nc.tensor.matmul(pv[:], g8[:, 2 * kp:2 * kp + 2, ts(to, 128)],
                                             wv8[:, 2 * kp:2 * kp + 2, ts(nb, 512)],
                                             start=(kp == 0), stop=(kp == 3),
                                             perf_mode=DR)
                        with nc.allow_low_precision(reason="fp8 v"):
                            nc.scalar.copy(
                                v_sb[:, to, nb * 8:(nb + 1) * 8, 0:64],
                                pv[:].rearrange("p (h d) -> p h d", d=64))

                if stage == "proj":
                    x2e = sb.tile([128, KO, Q], f32, tag="F", bufs=2, name="x2e")
                    nc.vector.tensor_tensor(x2e[:], qr[:], kr[:, :, 0:Q], ALU.add)
                    nc.vector.tensor_tensor(x2e[:, 0, :], x2e[:, 0, :], v_sb[:, 0, 0:4, 0:64].rearrange("p a b -> p (a b)")[:, 0:512], ALU.add)
                    nc.sync.dma_start(yt_d[:], x2e[:])
                    continue
                # ---- P3: attention ----
                oT8 = sb.tile([128, KO, Q], bf, tag="B", bufs=3, name="oT8")
                for hp in range(8):
                    jo = hp
                    probs = {0: [], 1: []}
                    for half in range(4):
                        pbig = {}
                        for sub in range(2):
                            r0 = sub * 64
                            big = psum2("sc")
                            for kk in range(2):
                                kt = half * 2 + kk
                                nc.tensor.matmul(big[:, ts(kk, 512)],
                                                 kr[r0:r0 + 64, jo, ts(kt, 128)],
                                                 qr[r0:r0 + 64, jo, :],
                                                 start=True, stop=True,
                                                 tile_position=(r0, 0))
                            pbig[sub] = big
                        for sub in range(2):
                            pb = sb.tile([128, 2, 512], f8, tag="probs", bufs=4,
                                         name="pb")
                            with nc.allow_low_precision(reason="fp8 probs"):
                                nc.scalar.activation(pb[:], pbig[sub][:], AF.Exp,
                                                     scale=0.125, bias=ln2_ap[:])
                            probs[sub].append(pb)
                    for sub in range(2):
                        h = 2 * hp + sub
                        r0 = sub * 64
                        po = psum("po")
                        for hf in range(4):
                            nc.tensor.matmul(po[0:65, :],
                                             v_sb[:, 2 * hf:2 * hf + 2, h, 0:65],
                                             probs[sub][hf][:],
                                             start=(hf == 0), stop=(hf == 3),
                                             perf_mode=DR)
                        rcp = sb.tile([65, 512], bf, tag="rcp", bufs=2, name="rcp")
                        with nc.allow_low_precision(reason="bf16 softmax denom"):
                            nc.vector.reciprocal(rcp[64:65, :], po[64:65, :])
                        prb = psum("prb")
                        nc.tensor.matmul(prb[0:64, :], ones_b[64:65, 0:64],
                                         rcp[64:65, :], start=True, stop=True)
                        rb = sb.tile([64, 512], bf, tag="rb", bufs=2, name="rb")
                        nc.vector.tensor_copy(rb[:], prb[0:64, :])
                        nc.vector.tensor_tensor(oT8[r0:r0 + 64, hp, :],
                                                po[0:64, :], rb[:], ALU.mult)

                if stage == "attn":
                    x2e = sb.tile([128, KO, Q], f32, tag="F", bufs=2, name="x2e2")
                    nc.vector.tensor_copy(x2e[:], oT8[:])
                    nc.sync.dma_start(yt_d[:], x2e[:])
                    continue
                # ---- P4: attn out + gated residual ----
                x2 = sb.tile([128, KO, Q], f32, tag="F", bufs=2, name="x2")
                for do in range(KO):
                    py = psum("py")
                    for ko in range(KO):
                        nc.tensor.matmul(py[:], wao_sb[do // 4][:, ko, ts(do % 4, 128)],
                                         oT8[:, ko, :], start=(ko == 0),
                                         stop=(ko == KO - 1))
                    nc.scalar.activation(x2[:, do, :], py[:], AF.Identity,
                                         bias=bo_s[:, do:do + 1],
                                         scale=gm_s[:, do:do + 1])
                # ---- P5: LN2 (512 tokens) ----
                x2b = sb.tile([128, KO, Q], bf, tag="B", bufs=3, name="x2b")
                sq2 = sb.tile([128, KO, Q], bf, tag="B", bufs=3, name="sq2")
                for hh in range(2):
                    c = slice(4 * hh, 4 * hh + 4)
                    nc.vector.tensor_tensor(x2[:, c, :], x2[:, c, :],
                                            xskip[:, c, :], ALU.add)
                    nc.scalar.copy(x2b[:, c, :], x2[:, c, :])
                    nc.scalar.square(sq2[:, c, :], x2[:, c, :])
                p1 = psum("lp1")
                p2 = psum("lp2")
                for ko in range(KO):
                    nc.tensor.matmul(p1[:], ones_b[:], x2b[:, ko, :],
                                     start=(ko == 0), stop=(ko == KO - 1))
                    nc.tensor.matmul(p2[:], ones_b[:], sq2[:, ko, :],
                                     start=(ko == 0), stop=(ko == KO - 1))
                mu = tmpf("mu2")
                nc.vector.tensor_scalar_mul(mu[:], p1[:], 1.0 / D)
                ex2 = tmpf("ex22")
                nc.vector.tensor_scalar_mul(ex2[:], p2[:], 1.0 / D)
                var = tmpf("var2")
                nc.vector.tensor_tensor(var[:], mu[:], mu[:], ALU.mult)
                nc.vector.tensor_tensor(ex2[:], ex2[:], var[:], ALU.subtract)
                nc.scalar.activation(var[:], ex2[:], AF.Sqrt, bias=eps_ap[:])
                mu16 = sb.tile([128, 512], bf, tag="stats16", bufs=2, name="mu16")
                nc.vector.tensor_copy(mu16[:], mu[:])
                rstd16 = sb.tile([128, 512], bf, tag="stats16", bufs=2, name="rstd16")
                with nc.allow_low_precision(reason="bf16 LN rstd"):
                    nc.vector.reciprocal(rstd16[:], var[:])
                pka2 = ps.tile([128, 512], f32, tag="p", bufs=4, name="pka2")
                for _ka in range(6):
                    nc.tensor.matmul(pka2[:], ones_b[:], mu16[:],
                                     start=(_ka == 0), stop=(_ka == 5))
                mu2_b = mu16[:].unsqueeze(1).broadcast_to([128, 4, Q])
                rstd2_b = rstd16[:].unsqueeze(1).broadcast_to([128, 4, Q])
                # in-place LN2 apply: x2b <- (x2b - mu) * rstd
                for hh in range(2):
                    c = slice(4 * hh, 4 * hh + 4)
                    nc.vector.tensor_tensor(x2b[:, c, :], x2b[:, c, :], mu2_b,
                                            ALU.subtract)
                    nc.vector.tensor_tensor(x2b[:, c, :], x2b[:, c, :],
                                            rstd2_b, ALU.mult)
                g2 = x2b

                if stage == "ln2":
                    nc.sync.dma_start(yt_d[:], x2[:])
                    continue
                # ---- P6/P7: MLP (bf16) ----
                m16 = sb.tile([128, 32, Q], bf, tag="m16", bufs=1)
                for mo in range(32):
                    pm = psum("pm")
                    for ko in range(KO):
                        nc.tensor.matmul(pm[:], w1_sb[mo // 4][:, ko, ts(mo % 4, 128)],
                                         g2[:, ko, :], start=(ko == 0),
                                         stop=(ko == KO - 1))
                    nc.scalar.activation(m16[:, mo], pm[:], AF.Gelu_apprx_tanh,
                                         bias=b1_s[:, mo:mo + 1], scale=1.0)
                for do in range(KO):
                    pz = psum("pz")
                    for ko in range(32):
                        nc.tensor.matmul(pz[:], w2_sb[do][:, ko, :],
                                         m16[:, ko, :], start=(ko == 0),
                                         stop=(ko == 31))
                    t = tmpf("t")
                    nc.scalar.activation(t[:], pz[:], AF.Identity,
                                         bias=b2_s[:, do:do + 1],
                                         scale=gp_s[:, do:do + 1])
                    nc.vector.tensor_tensor(x2[:, do, :], x2[:, do, :], t[:],
                                            ALU.add)
                    if do == 3:
                        nc.sync.dma_start(yt_d[:, 0:4, :], x2[:, 0:4, :])
                nc.sync.dma_start(yt_d[:, 4:8, :], x2[:, 4:8, :])

    nc.compile()
    return nc


# ----------------------------------------------------------------------------
# host wrapper
# ----------------------------------------------------------------------------

def _pieces(W, m_piece):
    """[K, M] weight -> [n_pieces, 128, K//128, m_piece], contiguous f32."""
    K, M = W.shape
    ko = K // 128
    Wr = np.asarray(W, np.float32).reshape(ko, 128, M).transpose(1, 0, 2)
    n = M // m_piece
    out = Wr.reshape(128, ko, n, m_piece).transpose(2, 0, 1, 3)
    return np.ascontiguousarray(out)


def _fold(W):
    """[K, M] -> [128, K//128, M] (partition-major chunks of the K axis)."""
    K, M = W.shape
    return np.ascontiguousarray(
        np.asarray(W, np.float32).reshape(K // 128, 128, M).transpose(1, 0, 2))


def _pvec(v):
    v = np.asarray(v, np.float32)
    return np.ascontiguousarray(v.reshape(-1, 128).T)


def _prep_shared(inputs):
    x = np.asarray(inputs["x"], np.float32)
    c = np.asarray(inputs["c"], np.float32)
    w_ada = np.asarray(inputs["w_ada"], np.float32)
    b_ada = np.asarray(inputs["b_ada"], np.float32)
    w_qkv = np.asarray(inputs["w_qkv"], np.float32)
    w_ao = np.asarray(inputs["w_attn_out"], np.float32)
    w_m1 = np.asarray(inputs["w_mlp1"], np.float32)
    w_m2 = np.asarray(inputs["w_mlp2"], np.float32)

    mod = c @ w_ada + b_ada
    sh_msa, sc_msa, g_msa, sh_mlp, sc_mlp, g_mlp = np.split(mod, 6, axis=1)
    ln1 = np.asarray(inputs["w_ln1"], np.float32) * (1.0 + sc_msa)
    ln2 = np.asarray(inputs["w_ln2"], np.float32) * (1.0 + sc_mlp)

    shared = {}
    for b in range(B):
        Wq = w_qkv[:, :D] * ln1[b][:, None]
        Wk = w_qkv[:, D:2 * D] * ln1[b][:, None]
        Wv = w_qkv[:, 2 * D:] * ln1[b][:, None]
        bqkv = sh_msa[b] @ w_qkv
        W1 = w_m1 * ln2[b][:, None]
        bm1 = sh_mlp[b] @ w_m1 + np.asarray(inputs["b_mlp1"], np.float32)
        biases = np.concatenate([
            _pvec(bqkv[:D]), _pvec(bqkv[D:2 * D]),
            _pvec((bqkv[2 * D:] @ w_ao) * g_msa[b]), _pvec(g_msa[b]),
            _pvec(bm1),
            _pvec(np.asarray(inputs["b_mlp2"], np.float32) * g_mlp[b]),
            _pvec(g_mlp[b]),
        ], axis=1).astype(np.float32)
        shared[b] = dict(
            wq8=np.ascontiguousarray(_fold(Wq).astype(F8)),
            wk8=np.ascontiguousarray(_fold(Wk).astype(F8)),
            wv8=np.ascontiguousarray(_fold(Wv).astype(F8)),
            wm1=np.ascontiguousarray(_pieces(W1, 512).astype(BF)),
            bias=np.ascontiguousarray(biases),
        )
    wao_p = np.ascontiguousarray(_pieces(w_ao, 512).astype(BF))
    wm2_p = np.ascontiguousarray(_pieces(w_m2, 128).astype(BF))
    cos = np.asarray(inputs["cos"], np.float32)
    sin = np.asarray(inputs["sin"], np.float32)

    sw = np.arange(128)
    sw = np.where(sw % 64 < 32, sw + 32, sw - 32)
    permw = np.zeros((128, 128), np.float32)
    permw[sw, np.arange(128)] = 1.0
    permw = np.ascontiguousarray(permw.astype(BF))
    return shared, wao_p, wm2_p, permw, x, cos, sin


def _make_in_maps(inputs):
    shared, wao_p, wm2_p, permw, x, cos, sin = _prep_shared(inputs)
    in_maps = []
    for core in range(8):
        b, half = core // 2, core % 2
        qlo = half * Q
        order = np.concatenate([np.arange(qlo, qlo + Q), np.arange(0, qlo),
                                np.arange(qlo + Q, S)])
        xT = x[b][order].T                       # [D, S]
        cosT = cos[order].T                      # [32, S]
        sinT = sin[order].T
        cc = np.concatenate([cosT] * 4, 0)
        ssm = np.concatenate([-sinT, sinT, -sinT, sinT], 0)
        ccss = np.ascontiguousarray(
            np.concatenate([cc, ssm], axis=1).astype(BF))   # [128, 2S]
        xf = xT.reshape(KO, 128, S).transpose(1, 0, 2)
        sh = shared[b]
        in_maps.append({
            "xb": np.ascontiguousarray(xf.astype(BF)),
            "xskip": np.ascontiguousarray(xf[:, :, :Q].astype(np.float32)),
            "ccss": ccss,
            "bias": sh["bias"],
            "permw": permw,
            "wq8": sh["wq8"], "wk8": sh["wk8"], "wv8": sh["wv8"],
            "wao": wao_p,
            "wm1": sh["wm1"], "wm2": wm2_p,
        })
    return in_maps


def kernel(**inputs):
    from concourse import bass_utils

    if "nc" not in _CACHE:
        _CACHE["nc"] = _build_program()
    nc = _CACHE["nc"]

    in_maps = _make_in_maps(inputs)
    res = bass_utils.run_bass_kernel_spmd(nc, in_maps, core_ids=list(range(8)))

    y = np.zeros((B, S, D), np.float32)
    for core in range(8):
        b, half = core // 2, core % 2
        qlo = half * Q
        yt = res.results[core]["yt"]             # [128, KO, Q]
        y[b, qlo:qlo + Q] = yt.transpose(1, 0, 2).reshape(D, Q).T
    return y
